# revision 9
# baseline (speedup 1.0000x reference)
"""Trainium2 Bass kernel for nn_MemoryOnGpu (retrieval_knn) — hybrid v3.

Per (query q, dataset d, bucket n): pick b* = argmax_b <q, key_db[b*128+n]>,
output that key and value row. One dataset per core.

The SWDGE indirect-DMA path costs ~1.4us per 128-row gather call on the Pool
engine (descgen is Pool-serial), so a full-gather kernel floors at ~1.45ms.
v3 splits each super-tile's buckets between two retrieval paths:
  - GSEL buckets/st: canonical indirect gathers (Pool), as in v2
  - the rest: on-chip PE selection — transpose the one-cold mask, +1 to
    one-hot while copying PSUM->SBUF, then sel = onehot^T @ kv2 (K=128 x2)
Scores: bf16 split-K ([ql;qh]@[kh;kl] one K=128 pass = both cross terms,
+ qh@kh K=64 second pass, PSUM-accumulated) = fp32-grade argmax accuracy.
Argmax: DVE segmented reduce-max (negated) -> ACT Sign(s+Mneg) in {-1,0}
-> (gather path) DVE STT (eq'+1)*biota accum-sum -> row index
-> (select path) PE transpose + ACT(+1 copy) + PE select matmuls.
"""

import sys

import numpy as np

for _p in ("/opt/trn_rl_repo", "/root/.axon_site/_ro/trn_rl_repo"):
    if _p not in sys.path:
        sys.path.insert(0, _p)

NUM_QUERIES = 1024
NUM_DATASETS = 8
DB_SIZE = 32768
KEY_FEATURES = 64
VALUE_FEATURES = 64
NUM_NEIGHBORS = 128

SW = 1024            # super-tile width (2 PSUM banks)
GSEL = 2             # buckets per super-tile on the gather path (of SW/256)
NSWQ = 4             # SWDGE queues for gather round-robin

_NC_CACHE = {}


def build_nc(Q=NUM_QUERIES, DB=DB_SIZE):
    import concourse.bass as bass
    import concourse.mybir as mybir
    import concourse.tile as tile
    from concourse import bacc

    F = KEY_FEATURES
    NB = NUM_NEIGHBORS
    BS = DB // NB               # 256 candidates per bucket
    KVW = 128                   # fp16 row: 64 key + 64 value
    NPST = SW // BS             # buckets per super-tile = 4
    NST = DB // SW              # super-tiles per q-chunk = 32
    QC = Q // 128               # q-chunks = 8
    NMM = SW // 512             # 512-col groups per super-tile = 2
    NSEL = NPST - GSEL          # PE-selected buckets per super-tile
    assert 0 < GSEL < NPST

    nc = bacc.Bacc(num_swdge_queues=NSWQ)
    # scores split-K operands: [ql; qh] stationary, [kh; kl] moving
    qlh = nc.declare_dram_parameter("qlh", [2 * F, Q], mybir.dt.bfloat16, isOutput=False)
    khkl = nc.declare_dram_parameter("khkl", [2 * F, DB], mybir.dt.bfloat16, isOutput=False)
    kv = nc.declare_dram_parameter("kv", [DB, KVW], mybir.dt.float16, isOutput=False)
    # kv2[p, n*2+bh, f] = kv[(bh*128+p)*128 + n, f]  (select-path table)
    kv2 = nc.declare_dram_parameter("kv2", [128, 2 * NB * KVW], mybir.dt.float16, isOutput=False)
    biota = nc.declare_dram_parameter("biota", [128, SW], mybir.dt.float16, isOutput=False)
    niota = nc.declare_dram_parameter("niota", [128, NB], mybir.dt.float32, isOutput=False)
    ident = nc.declare_dram_parameter("ident", [128, 128], mybir.dt.float16, isOutput=False)
    okv = nc.declare_dram_parameter("okv", [Q, NB, KVW], mybir.dt.float16, isOutput=True)

    X = mybir.AxisListType.X
    OP = mybir.AluOpType
    AF = mybir.ActivationFunctionType

    with tile.TileContext(nc) as tc:
        with (
            tc.tile_pool(name="const", bufs=1) as constp,
            tc.tile_pool(name="eqs", bufs=6) as eqp,
            tc.tile_pool(name="tr", bufs=3) as trp,
            tc.tile_pool(name="sel", bufs=6) as selp,
            tc.tile_pool(name="onh", bufs=6) as onhp,
            tc.tile_pool(name="selo", bufs=6) as selop,
            tc.tile_pool(name="acc", bufs=2) as accp,
            tc.tile_pool(name="gkv", bufs=8) as gkvp,
            tc.tile_pool(name="ps", bufs=3, space="PSUM") as psp,
            tc.tile_pool(name="pt", bufs=1, space="PSUM") as ptp,
            tc.tile_pool(name="po", bufs=1, space="PSUM") as pop,
        ):
            qt = constp.tile([2 * F, Q], mybir.dt.bfloat16, tag="qt")
            nc.sync.dma_start(out=qt[:], in_=qlh[:])
            qh2 = constp.tile([F, Q], mybir.dt.bfloat16, tag="qh2")
            nc.sync.dma_start(out=qh2[:], in_=qlh[F:2 * F, :])
            kt = constp.tile([2 * F, DB], mybir.dt.bfloat16, tag="kt")
            for c in range(8):
                w = DB // 8
                nc.sync.dma_start(out=kt[:, c * w:(c + 1) * w],
                                  in_=khkl[:, c * w:(c + 1) * w])
            k2 = constp.tile([128, 2 * NB * KVW], mybir.dt.float16, tag="k2")
            for c in range(8):
                w = 2 * NB * KVW // 8
                nc.sync.dma_start(out=k2[:, c * w:(c + 1) * w],
                                  in_=kv2[:, c * w:(c + 1) * w])
            bio = constp.tile([128, SW], mybir.dt.float16, tag="bio")
            nc.sync.dma_start(out=bio[:], in_=biota[:])
            nio = constp.tile([128, NB], mybir.dt.float32, tag="nio")
            nc.sync.dma_start(out=nio[:], in_=niota[:])
            idt = constp.tile([128, 128], mybir.dt.float16, tag="idt")
            nc.sync.dma_start(out=idt[:], in_=ident[:])

            gq = 0
            for qc in range(QC):
                Mn = accp.tile([128, NB], mybir.dt.float32, tag="Mn")
                qs = qt[:, qc * 128:(qc + 1) * 128]
                qh_only = qh2[:, qc * 128:(qc + 1) * 128]
                for st in range(NST):
                    gsel = 3 if st % 4 == 0 else GSEL
                    nsel = NPST - gsel
                    ps = psp.tile([128, SW], mybir.dt.float32, tag="ps")
                    for j in range(NMM):
                        c0 = st * SW + j * 512
                        # pass 1 (K=128): ql@kh + qh@kl
                        nc.tensor.matmul(
                            ps[:, j * 512:(j + 1) * 512], qs,
                            kt[:, c0:c0 + 512], start=True, stop=False,
                        )
                        # pass 2 (K=64): qh@kh
                        nc.tensor.matmul(
                            ps[:, j * 512:(j + 1) * 512], qh_only,
                            kt[0:F, c0:c0 + 512], start=False, stop=True,
                        )
                    n0 = st * NPST
                    nc.vector.tensor_reduce(
                        Mn[:, n0:n0 + NPST],
                        ps[:].rearrange("p (n b) -> p n b", b=BS),
                        axis=X, op=OP.max, negate=True,
                    )
                    eq = eqp.tile([128, SW], mybir.dt.float16, tag="eq")
                    for nl in list(range(gsel, NPST)) + list(range(gsel)):
                        nc.scalar.activation(
                            out=eq[:, nl * BS:(nl + 1) * BS],
                            in_=ps[:, nl * BS:(nl + 1) * BS],
                            func=AF.Sign,
                            bias=Mn[:, n0 + nl:n0 + nl + 1],
                        )
                    # ---- gather path: buckets n0 .. n0+GSEL-1 ----
                    bsel = selp.tile([128, 3], mybir.dt.float32, tag="bsel")
                    trash = trp.tile([128, BS], mybir.dt.float16, tag="trash")
                    for nl in range(gsel):
                        nc.vector.scalar_tensor_tensor(
                            out=trash[:],
                            in0=eq[:, nl * BS:(nl + 1) * BS],
                            scalar=1.0,
                            in1=bio[:, nl * BS:(nl + 1) * BS],
                            op0=OP.add, op1=OP.mult,
                            accum_out=bsel[:, nl:nl + 1],
                        )
                    offf = selp.tile([128, 3], mybir.dt.float32, tag="offf")
                    nc.vector.scalar_tensor_tensor(
                        out=offf[:, 0:gsel], in0=bsel[:, 0:gsel], scalar=32640.0,
                        in1=nio[:, n0:n0 + gsel],
                        op0=OP.min, op1=OP.add,
                    )
                    offi = selp.tile([128, 3], mybir.dt.int32, tag="offi")
                    nc.vector.tensor_copy(out=offi[:, 0:gsel], in_=offf[:, 0:gsel])
                    gk = gkvp.tile([128, 3 * KVW], mybir.dt.float16, tag="gk")
                    for nl in range(gsel):
                        gi = nc.gpsimd.indirect_dma_start(
                            out=gk[:, nl * KVW:(nl + 1) * KVW],
                            out_offset=None,
                            in_=kv[:],
                            in_offset=bass.IndirectOffsetOnAxis(
                                ap=offi[:, nl:nl + 1], axis=0
                            ),
                        )
                        gi.ins.queue = f"qPoolDynamic{gq or ''}"
                        gq = (gq + 1) % NSWQ
                    nc.sync.dma_start(
                        out=okv[qc * 128:(qc + 1) * 128, n0:n0 + gsel, :],
                        in_=gk[:, 0:gsel * KVW].rearrange("p (g f) -> p g f", f=KVW),
                    )
                    # ---- select path: buckets n0+GSEL .. n0+NPST-1 ----
                    eqT = ptp.tile([128, NSEL * BS], mybir.dt.float16, tag="eqT")
                    for si in range(nsel):
                        nl = gsel + si
                        for bh in range(2):
                            nc.tensor.transpose(
                                eqT[:, (si * 2 + bh) * 128:(si * 2 + bh + 1) * 128],
                                eq[:, nl * BS + bh * 128:nl * BS + (bh + 1) * 128],
                                idt[:],
                            )
                    onh = onhp.tile([128, NSEL * BS], mybir.dt.float16, tag="onh")
                    w = nsel * BS
                    if st % 2 == 0:
                        nc.scalar.activation(
                            out=onh[:, 0:w], in_=eqT[:, 0:w], func=AF.Identity, bias=1.0,
                        )
                    else:
                        nc.vector.tensor_scalar(
                            out=onh[:, 0:w], in0=eqT[:, 0:w], scalar1=1.0, scalar2=None,
                            op0=OP.add,
                        )
                    selps = pop.tile([128, NSEL * KVW], mybir.dt.float32, tag="selps")
                    for si in range(nsel):
                        n = n0 + gsel + si
                        for bh in range(2):
                            nc.tensor.matmul(
                                selps[:, si * KVW:(si + 1) * KVW],
                                onh[:, (si * 2 + bh) * 128:(si * 2 + bh + 1) * 128],
                                k2[:, (n * 2 + bh) * KVW:(n * 2 + bh + 1) * KVW],
                                start=(bh == 0), stop=(bh == 1),
                            )
                    selo = selop.tile([128, NSEL * KVW], mybir.dt.float16, tag="selo")
                    w2 = nsel * KVW
                    if st % 2 == 0:
                        nc.scalar.activation(out=selo[:, 0:w2], in_=selps[:, 0:w2], func=AF.Copy)
                    else:
                        nc.vector.tensor_copy(out=selo[:, 0:w2], in_=selps[:, 0:w2])
                    nc.sync.dma_start(
                        out=okv[qc * 128:(qc + 1) * 128, n0 + gsel:n0 + NPST, :],
                        in_=selo[:, 0:w2].rearrange("p (g f) -> p g f", f=KVW),
                    )
    if not nc.is_finalized():
        nc.finalize()
    return nc


def _get_nc(Q, DB):
    key = (Q, DB)
    if key not in _NC_CACHE:
        _NC_CACHE[key] = build_nc(Q, DB)
    return _NC_CACHE[key]


def make_core_inputs(query, key_db, value_db, d, Q, DB):
    """Host-side prep of one core's input arrays (dataset d)."""
    import ml_dtypes

    F = KEY_FEATURES
    NB = NUM_NEIGHBORS
    BS = DB // NB
    KVW = 128
    bf16 = ml_dtypes.bfloat16
    qTn = query[:, d, :].T.astype(np.float32)                                 # (F, Q)
    qh = qTn.astype(bf16)
    ql = (qTn - qh.astype(np.float32)).astype(bf16)
    qlh = np.ascontiguousarray(np.concatenate([ql, qh], axis=0))              # (2F, Q)
    kperm = key_db[d].reshape(BS, NB, F).transpose(2, 1, 0)                   # (F, NB, BS)
    kTpn = kperm.reshape(F, NB * BS).astype(np.float32)
    kh = kTpn.astype(bf16)
    kl = (kTpn - kh.astype(np.float32)).astype(bf16)
    khkl = np.ascontiguousarray(np.concatenate([kh, kl], axis=0))             # (2F, DB)
    kvn = np.ascontiguousarray(
        np.concatenate([key_db[d], value_db[d]], axis=1).astype(np.float16)  # (DB, 128)
    )
    # kv2[p, n*2+bh, f] = kvn[(bh*128+p)*128 + n, f]
    kv2 = kvn.reshape(2, 128, NB, KVW)            # (bh, p, n, f)
    kv2 = np.ascontiguousarray(
        kv2.transpose(1, 2, 0, 3).reshape(128, 2 * NB * KVW)
    )
    bio01 = ((np.arange(SW) % BS).astype(np.float32) * 128.0).astype(np.float16)
    bio = np.broadcast_to(bio01, (128, SW)).copy()
    nio = np.broadcast_to(np.arange(NB, dtype=np.float32), (128, NB)).copy()
    identm = np.eye(128, dtype=np.float16)
    return {"qlh": qlh, "khkl": khkl, "kv": kvn, "kv2": kv2, "biota": bio,
            "niota": nio, "ident": identm}


def kernel(query, key_db, value_db, num_neighbors):
    from concourse.bass_utils import run_bass_kernel_spmd

    query = np.asarray(query, dtype=np.float32)
    key_db = np.asarray(key_db, dtype=np.float32)
    value_db = np.asarray(value_db, dtype=np.float32)
    assert int(num_neighbors) == NUM_NEIGHBORS
    Q, D, F = query.shape
    _, DB, _ = key_db.shape
    assert (Q, D, F, DB) == (NUM_QUERIES, NUM_DATASETS, KEY_FEATURES, DB_SIZE)

    nc = _get_nc(Q, DB)
    in_maps = [make_core_inputs(query, key_db, value_db, d, Q, DB) for d in range(D)]
    res = run_bass_kernel_spmd(nc, in_maps, core_ids=list(range(D)))

    sel_k = np.empty((Q, D, NUM_NEIGHBORS, KEY_FEATURES), dtype=np.float32)
    sel_v = np.empty((Q, D, NUM_NEIGHBORS, VALUE_FEATURES), dtype=np.float32)
    for d in range(D):
        okv = np.asarray(res.results[d]["okv"], dtype=np.float32)
        sel_k[:, d] = okv[:, :, :KEY_FEATURES]
        sel_v[:, d] = okv[:, :, KEY_FEATURES:]
    return sel_k, sel_v
